# revision 4
# baseline (speedup 1.0000x reference)
"""Trainium2 Bass kernel for a 2-layer ReLU RNN (batch_first) + linear head.

Problem shapes: B=256, T=512, I=512, H=1024, O=256 (fp32).
Sharding: data-parallel over batch across 8 NeuronCores (32 rows each);
weights replicated. No collectives needed.

Per-core dataflow (all matmuls fp32r = full-rate reduced-precision fp32):
  Phase A: pre0T = W_ih0 @ x^T + biases       (stationary weight tiles, tokens stream)
  Phase B: layer-0 recurrence over T steps:
             state kept TRANSPOSED: hT[jj, kt*32+b] = h[b, 128*kt+jj]
             s = h @ W_hh.T via stationary-hT matmuls -> psum [32, 1024]
             hT_next = relu(transpose(s) + preT[t]); transpose on the PE
             (stationary = s-chunk [32,128], rhs = 32x32 identity)
             hT dumped to DRAM each step (input of phase C)
  Phase C: pre1T = W_ih1 @ h0^T + biases      (like phase A)
  Phase D: layer-1 recurrence (like B, no dump)
  Phase E: out = h1[:, -1, :] @ fc_w.T + fc_b

kernel(**inputs) takes the FULL unsharded inputs (keys as in reference
setup_inputs) and returns the FULL [256, 256] output.
"""

import numpy as np

import concourse.bass as bass
import concourse.tile as tile
import concourse.mybir as mybir
from concourse import bacc
from concourse.bass_utils import run_bass_kernel_spmd

F32 = mybir.dt.float32
F32R = mybir.dt.float32r

B_FULL, T_FULL, I_DIM, H, O = 256, 512, 512, 1024, 256
N_CORES = 8
BL = B_FULL // N_CORES  # 32 batch rows per core
KI = I_DIM // 128       # 4  k-tiles of the input dim
KT = H // 128           # 8  k-tiles of the hidden dim


def _phase_proj(nc, tc, ntok, n_ki, w_d, bias_sb, rhs_load, pre_dram, name, rhs_bufs):
    """pre^T[j, tok] = sum_i W[j, i] * rhs[i, tok] + bias[j], streamed over
    512-token chunks, 8 psum banks in flight.

    w_d: DRAM [128, n_ki*H] fp32r, [r, ki*H + j] = W[j, 128*ki + r]
    rhs_load(tci, dest): emits DMA filling dest [128, n_ki*512] with
             rhs[128*ki + r, 512*tci + n] at [r, ki*512 + n]
    pre_dram: [T, 128, 256] fp32; [t, jj, 32*m + b] = pre[b, 128*m + jj]
    """
    n_tc = ntok // 512
    with (
        tc.tile_pool(name=f"{name}_w", bufs=1) as w_pool,
        tc.tile_pool(name=f"{name}_rhs", bufs=rhs_bufs) as rhs_pool,
        tc.tile_pool(name=f"{name}_ps", bufs=1, space="PSUM") as ps_pool,
        tc.tile_pool(name=f"{name}_st", bufs=4) as st_pool,
    ):
        w_sb = w_pool.tile([128, n_ki * H], F32R)
        nc.sync.dma_start(w_sb[:], w_d)
        for tcg in range(0, n_tc, 8):
            chunk = list(range(tcg, min(tcg + 8, n_tc)))
            rhs_tiles = []
            for tci in chunk:
                rt = rhs_pool.tile([128, n_ki * 512], F32R, tag="rhs")
                rhs_load(tci, rt)
                rhs_tiles.append(rt)
            for m in range(KT):
                for sl, (tci, rt) in enumerate(zip(chunk, rhs_tiles)):
                    ps = ps_pool.tile([128, 512], F32, tag=f"ps{sl}")
                    for ki in range(n_ki):
                        nc.tensor.matmul(
                            ps[:, :],
                            w_sb[:, ki * H + 128 * m : ki * H + 128 * m + 128],
                            rt[:, ki * 512 : ki * 512 + 512],
                            start=(ki == 0),
                            stop=(ki == n_ki - 1),
                        )
                    st = st_pool.tile([128, 512], F32, tag="st")
                    nc.scalar.activation(
                        st[:, :], ps[:, :],
                        mybir.ActivationFunctionType.Identity,
                        bias=bias_sb[:, m : m + 1],
                    )
                    # dest: pre_dram[t0 + tt, jj, 32*m + b], 16 t per chunk
                    t0 = tci * 512 // BL
                    nc.sync.dma_start(
                        pre_dram[t0 : t0 + 16, :, 32 * m : 32 * m + 32]
                        .rearrange("t p b -> p t b"),
                        st[:, :].rearrange("p (t b) -> p t b", b=BL),
                    )


def _phase_recur(nc, tc, T, hT_pool, whh_d, i32_sb, pre_dram, hT_dump, name, zeros_d):
    """Recurrence; returns final hT tile (allocated from caller-owned hT_pool).

    whh_d: DRAM [128, KT*H] fp32r, [r, kt*H + j] = W_hh[j, 128*kt + r]
    pre_dram: [T, 128, 256] fp32;  hT_dump: None or DRAM [T, 128, 256] f32r
    """
    with (
        tc.tile_pool(name=f"{name}_w", bufs=1) as w_pool,
        tc.tile_pool(name=f"{name}_pr", bufs=4) as pr_pool,
        tc.tile_pool(name=f"{name}_ssb", bufs=3) as ssb_pool,
        tc.tile_pool(name=f"{name}_tmp", bufs=4) as tmp_pool,
        tc.tile_pool(name=f"{name}_ps", bufs=2, space="PSUM") as ps_pool,
        tc.tile_pool(name=f"{name}_ps2", bufs=2, space="PSUM") as ps2_pool,
    ):
        whh_sb = w_pool.tile([128, KT * H], F32R)
        nc.sync.dma_start(whh_sb[:], whh_d)
        hT = hT_pool.tile([128, 2 * 128], F32R, tag="hT")
        nc.sync.dma_start(hT[:, :], zeros_d)
        for t in range(T):
            pr = pr_pool.tile([128, 256], F32, tag="pr")
            nc.sync.dma_start(
                pr[:, :].rearrange("p (t b) -> p t b", t=1),
                pre_dram[t : t + 1, :, :].rearrange("t p b -> p t b"),
            )
            s_sb = ssb_pool.tile([BL, H], F32R, tag="ssb")
            for nh in range(2):
                sp = ps_pool.tile([BL, 512], F32, tag=f"s{nh}")
                for kt in range(KT):
                    nc.tensor.matmul(
                        sp[:, :],
                        hT[:, 32 * kt : 32 * kt + 32],
                        whh_sb[:, kt * H + 512 * nh : kt * H + 512 * nh + 512],
                        start=(kt == 0),
                        stop=(kt == KT - 1),
                    )
                nc.vector.tensor_copy(s_sb[:, 512 * nh : 512 * nh + 512], sp[:, :])
            hT_next = hT_pool.tile([128, 2 * 128], F32R, tag="hT")
            for nh in range(2):
                o2 = ps2_pool.tile([128, 128], F32, tag=f"o2{nh}")
                for c in range(4):
                    nc.tensor.matmul(
                        o2[:, 32 * c : 32 * c + 32],
                        s_sb[:, 512 * nh + 128 * c : 512 * nh + 128 * c + 128],
                        i32_sb[:, :],
                        start=(c == 0),
                        stop=(c == 3),
                        skip_group_check=True,
                    )
                tmp = tmp_pool.tile([128, 128], F32, tag=f"tmp{nh}")
                nc.vector.tensor_add(
                    tmp[:, :], o2[:, :], pr[:, 128 * nh : 128 * nh + 128]
                )
                nc.scalar.activation(
                    hT_next[:, 128 * nh : 128 * nh + 128],
                    tmp[:, :],
                    mybir.ActivationFunctionType.Relu,
                )
            if hT_dump is not None:
                nc.sync.dma_start(
                    hT_dump[t : t + 1, :, :].rearrange("t p b -> p t b"),
                    hT_next[:, :].rearrange("p (t b) -> p t b", t=1),
                )
            hT = hT_next
        return hT


def build_rnn(T):
    ntok = T * BL
    nc = bacc.Bacc("TRN2", target_bir_lowering=False, debug=False)

    xT_d = nc.dram_tensor("xT", [I_DIM, ntok], F32R, kind="ExternalInput").ap()
    wih0_d = nc.dram_tensor("wih0T", [128, KI * H], F32R, kind="ExternalInput").ap()
    whh0_d = nc.dram_tensor("whh0T", [128, KT * H], F32R, kind="ExternalInput").ap()
    wih1_d = nc.dram_tensor("wih1T", [128, KT * H], F32R, kind="ExternalInput").ap()
    whh1_d = nc.dram_tensor("whh1T", [128, KT * H], F32R, kind="ExternalInput").ap()
    fcw_d = nc.dram_tensor("fcwT", [128, KT * O], F32R, kind="ExternalInput").ap()
    bias0_d = nc.dram_tensor("bias0", [128, KT], F32, kind="ExternalInput").ap()
    bias1_d = nc.dram_tensor("bias1", [128, KT], F32, kind="ExternalInput").ap()
    fcb_d = nc.dram_tensor("fcb", [BL, O], F32, kind="ExternalInput").ap()
    i32_d = nc.dram_tensor("i32", [BL, BL], F32R, kind="ExternalInput").ap()
    zeros_d = nc.dram_tensor("zeros", [128, 2 * 128], F32R, kind="ExternalInput").ap()
    out_d = nc.dram_tensor("out", [BL, O], F32, kind="ExternalOutput").ap()

    with tile.TileContext(nc) as tc:
        with (
            tc.tile_pool(name="dram", bufs=1, space="DRAM") as dram_pool,
            tc.tile_pool(name="const", bufs=1) as cpool,
            tc.tile_pool(name="hT", bufs=2) as hT_pool,
        ):
            pre0_dram = dram_pool.tile([T, 128, 2 * 128], F32)
            pre1_dram = dram_pool.tile([T, 128, 2 * 128], F32)
            h0T_dram = dram_pool.tile([T, 128, 2 * 128], F32R)

            bias0_sb = cpool.tile([128, KT], F32)
            bias1_sb = cpool.tile([128, KT], F32)
            i32_sb = cpool.tile([BL, BL], F32R)
            fcb_sb = cpool.tile([BL, O], F32)
            nc.sync.dma_start(bias0_sb[:], bias0_d)
            nc.sync.dma_start(bias1_sb[:], bias1_d)
            nc.sync.dma_start(i32_sb[:], i32_d)
            nc.sync.dma_start(fcb_sb[:], fcb_d)

            # ---------- Phase A: pre0T ----------
            def load_x(tci, dest):
                nc.sync.dma_start(
                    dest[:, :].rearrange("p (ki n) -> p ki n", ki=KI),
                    xT_d[:, 512 * tci : 512 * tci + 512]
                    .rearrange("(ki p) n -> p ki n", p=128),
                )

            _phase_proj(nc, tc, ntok, KI, wih0_d, bias0_sb, load_x,
                        pre0_dram, "pA", rhs_bufs=9)

            # ---------- Phase B: layer-0 recurrence ----------
            _phase_recur(nc, tc, T, hT_pool, whh0_d, i32_sb, pre0_dram,
                         h0T_dram, "pB", zeros_d)

            # ---------- Phase C: pre1T ----------
            def load_h0(tci, dest):
                # dest[r, kt*512 + tt*32 + b] = h0T[t0+tt, r, 32*kt + b]
                t0 = tci * 512 // BL
                nc.sync.dma_start(
                    dest[:, :].rearrange("p (kt tt b) -> p kt tt b", kt=KT, b=BL),
                    h0T_dram[t0 : t0 + 16, :, :]
                    .rearrange("tt p (kt b) -> p kt tt b", b=BL),
                )

            _phase_proj(nc, tc, ntok, KT, wih1_d, bias1_sb, load_h0,
                        pre1_dram, "pC", rhs_bufs=8)

            # ---------- Phase D: layer-1 recurrence ----------
            hT_fin = _phase_recur(nc, tc, T, hT_pool, whh1_d, i32_sb,
                                  pre1_dram, None, "pD", zeros_d)

            # ---------- Phase E: head ----------
            with (
                tc.tile_pool(name="fcw", bufs=1) as fpool,
                tc.tile_pool(name="eps", bufs=1, space="PSUM") as eps_pool,
                tc.tile_pool(name="eout", bufs=1) as eo_pool,
            ):
                fcw_sb = fpool.tile([128, KT * O], F32R)
                nc.sync.dma_start(fcw_sb[:], fcw_d)
                ep = eps_pool.tile([BL, O], F32)
                for kt in range(KT):
                    nc.tensor.matmul(
                        ep[:, :],
                        hT_fin[:, 32 * kt : 32 * kt + 32],
                        fcw_sb[:, kt * O : kt * O + O],
                        start=(kt == 0),
                        stop=(kt == KT - 1),
                    )
                eo = eo_pool.tile([BL, O], F32)
                nc.vector.tensor_add(eo[:, :], ep[:, :], fcb_sb[:, :])
                nc.sync.dma_start(out_d, eo[:, :])

    nc.compile()
    return nc


def _prep_core_inputs(inputs, T):
    """Host-side prep: transposed weights (shared) + per-core xT shards."""
    f32 = np.float32
    W_ih0 = np.asarray(inputs["W_ih0"], f32)
    W_hh0 = np.asarray(inputs["W_hh0"], f32)
    W_ih1 = np.asarray(inputs["W_ih1"], f32)
    W_hh1 = np.asarray(inputs["W_hh1"], f32)
    fc_w = np.asarray(inputs["fc_w"], f32)

    def stack_T(W, n_k):  # [128, n_k*cols]: [r, k*cols + j] = W[j, 128k + r]
        cols = W.shape[0]
        out = np.empty((128, n_k * cols), f32)
        WT = np.ascontiguousarray(W.T)  # [in, out]
        for k in range(n_k):
            out[:, k * cols : (k + 1) * cols] = WT[128 * k : 128 * (k + 1), :]
        return out

    shared = {
        "wih0T": stack_T(W_ih0, KI),
        "whh0T": stack_T(W_hh0, KT),
        "wih1T": stack_T(W_ih1, KT),
        "whh1T": stack_T(W_hh1, KT),
        "fcwT": stack_T(fc_w, KT),
        "bias0": np.ascontiguousarray(
            (np.asarray(inputs["b_ih0"], f32) + np.asarray(inputs["b_hh0"], f32))
            .reshape(KT, 128).T),
        "bias1": np.ascontiguousarray(
            (np.asarray(inputs["b_ih1"], f32) + np.asarray(inputs["b_hh1"], f32))
            .reshape(KT, 128).T),
        "fcb": np.tile(np.asarray(inputs["fc_b"], f32)[None, :], (BL, 1)),
        "i32": np.eye(BL, dtype=f32),
        "zeros": np.zeros((128, 256), f32),
    }
    x = np.asarray(inputs["input_data"], f32)  # [B, T, I]
    in_maps = []
    for c in range(N_CORES):
        xs = x[c * BL : (c + 1) * BL, :T, :]          # [BL, T, I]
        xT = np.ascontiguousarray(np.transpose(xs, (2, 1, 0))).reshape(I_DIM, T * BL)
        in_maps.append(dict(shared, xT=xT))
    return in_maps


def run(inputs, trace=False, trace_kwargs=None):
    T = np.asarray(inputs["input_data"]).shape[1]
    nc = build_rnn(T)
    in_maps = _prep_core_inputs(inputs, T)
    res = run_bass_kernel_spmd(
        nc, in_maps, list(range(N_CORES)), trace=trace, **(trace_kwargs or {})
    )
    out = np.concatenate([res.results[c]["out"] for c in range(N_CORES)], axis=0)
    return out, res


def kernel(**inputs):
    return run(inputs)[0]


# revision 7
# speedup vs baseline: 1.0991x; 1.0991x over previous
"""Trainium2 Bass kernel for a 2-layer ReLU RNN (batch_first) + linear head.

Problem shapes: B=256, T=512, I=512, H=1024, O=256 (fp32).
Sharding: data-parallel over batch across 8 NeuronCores (32 rows each);
weights replicated. No collectives needed.

Per-core dataflow (all matmuls fp32r = full-rate reduced-precision fp32):
  Phase A: pre0T = W_ih0 @ x^T + biases       (stationary weight tiles, tokens stream)
  Phase B: layer-0 recurrence over T steps:
             state kept TRANSPOSED: hT[jj, kt*32+b] = h[b, 128*kt+jj]
             s = h @ W_hh.T via stationary-hT matmuls -> psum [32, 1024]
             hT_next = relu(transpose(s) + preT[t]); transpose on the PE
             (stationary = s-chunk [32,128], rhs = 32x32 identity)
             hT dumped to DRAM each step (input of phase C)
  Phase C: pre1T = W_ih1 @ h0^T + biases      (like phase A)
  Phase D: layer-1 recurrence (like B, no dump)
  Phase E: out = h1[:, -1, :] @ fc_w.T + fc_b

kernel(**inputs) takes the FULL unsharded inputs (keys as in reference
setup_inputs) and returns the FULL [256, 256] output.
"""

import ml_dtypes
import numpy as np

import concourse.bass as bass
import concourse.tile as tile
import concourse.mybir as mybir
from concourse import bacc
from concourse.bass_utils import run_bass_kernel_spmd

F32 = mybir.dt.float32
F32R = mybir.dt.float32r
BF16 = mybir.dt.bfloat16

B_FULL, T_FULL, I_DIM, H, O = 256, 512, 512, 1024, 256
N_CORES = 8
BL = B_FULL // N_CORES  # 32 batch rows per core
KI = I_DIM // 128       # 4  k-tiles of the input dim
KT = H // 128           # 8  k-tiles of the hidden dim


def _phase_proj(nc, tc, ntok, n_ki, w_d, bias_sb, rhs_load, pre_dram, name, rhs_bufs):
    """pre^T[j, tok] = sum_i W[j, i] * rhs[i, tok] + bias[j], streamed over
    512-token chunks, 8 psum banks in flight.

    w_d: DRAM [128, n_ki*H] fp32r, [r, ki*H + j] = W[j, 128*ki + r]
    rhs_load(tci, dest): emits DMA filling dest [128, n_ki*512] with
             rhs[128*ki + r, 512*tci + n] at [r, ki*512 + n]
    pre_dram: [T, 128, 256] fp32; [t, jj, 32*m + b] = pre[b, 128*m + jj]
    """
    n_tc = ntok // 512
    with (
        tc.tile_pool(name=f"{name}_w", bufs=1) as w_pool,
        tc.tile_pool(name=f"{name}_rhs", bufs=rhs_bufs) as rhs_pool,
        tc.tile_pool(name=f"{name}_ps", bufs=1, space="PSUM") as ps_pool,
        tc.tile_pool(name=f"{name}_st", bufs=4) as st_pool,
    ):
        w_sb = w_pool.tile([128, n_ki * H], F32R)
        nc.sync.dma_start(w_sb[:], w_d)
        for tcg in range(0, n_tc, 8):
            chunk = list(range(tcg, min(tcg + 8, n_tc)))
            rhs_tiles = []
            for tci in chunk:
                rt = rhs_pool.tile([128, n_ki * 512], F32R, tag="rhs")
                rhs_load(tci, rt)
                rhs_tiles.append(rt)
            for m in range(KT):
                for sl, (tci, rt) in enumerate(zip(chunk, rhs_tiles)):
                    ps = ps_pool.tile([128, 512], F32, tag=f"ps{sl}")
                    for ki in range(n_ki):
                        nc.tensor.matmul(
                            ps[:, :],
                            w_sb[:, ki * H + 128 * m : ki * H + 128 * m + 128],
                            rt[:, ki * 512 : ki * 512 + 512],
                            start=(ki == 0),
                            stop=(ki == n_ki - 1),
                        )
                    st = st_pool.tile([128, 512], F32, tag="st")
                    nc.scalar.activation(
                        st[:, :], ps[:, :],
                        mybir.ActivationFunctionType.Identity,
                        bias=bias_sb[:, m : m + 1],
                    )
                    # dest: pre_dram[t0 + tt, jj, 32*m + b], 16 t per chunk
                    t0 = tci * 512 // BL
                    nc.sync.dma_start(
                        pre_dram[t0 : t0 + 16, :, 32 * m : 32 * m + 32]
                        .rearrange("t p b -> p t b"),
                        st[:, :].rearrange("p (t b) -> p t b", b=BL),
                    )


def _phase_recur(nc, tc, T, hT_pool, whh_d, i32_sb, pre_load, hT_store, name, zeros_d, on_step=None):
    """Recurrence; returns final hT tile (allocated from caller-owned hT_pool).

    whh_d: DRAM [128, KT*H] fp32r, [r, kt*H + j] = W_hh[j, 128*kt + r]
    pre_load(t, pr): DMA step-t preT into pr [128, 256]
    hT_store(t, hT): optional per-step dump;  on_step(t): post-step hook
    """
    with (
        tc.tile_pool(name=f"{name}_w", bufs=1) as w_pool,
        tc.tile_pool(name=f"{name}_pr", bufs=4) as pr_pool,
        tc.tile_pool(name=f"{name}_ssb", bufs=3) as ssb_pool,
        tc.tile_pool(name=f"{name}_tmp", bufs=4) as tmp_pool,
        tc.tile_pool(name=f"{name}_ps", bufs=1, space="PSUM") as ps_pool,
        tc.tile_pool(name=f"{name}_ps2", bufs=1, space="PSUM") as ps2_pool,
    ):
        whh_sb = w_pool.tile([128, KT * H], F32R)
        nc.sync.dma_start(whh_sb[:], whh_d)
        hT = hT_pool.tile([128, 2 * 128], F32R, tag="hT")
        nc.sync.dma_start(hT[:, :], zeros_d)
        for t in range(T):
            pr = pr_pool.tile([128, 256], F32, tag="pr")
            pre_load(t, pr)
            s_sb = ssb_pool.tile([BL, H], BF16, tag="ssb")
            for nh in range(2):
                sp = ps_pool.tile([BL, 512], F32, tag=f"s{nh}")
                for kt in range(KT):
                    nc.tensor.matmul(
                        sp[:, :],
                        hT[:, 32 * kt : 32 * kt + 32],
                        whh_sb[:, kt * H + 512 * nh : kt * H + 512 * nh + 512],
                        start=(kt == 0),
                        stop=(kt == KT - 1),
                    )
                nc.vector.tensor_copy(s_sb[:, 512 * nh : 512 * nh + 512], sp[:, :])
            hT_next = hT_pool.tile([128, 2 * 128], F32R, tag="hT")
            for nh in range(2):
                o2 = ps2_pool.tile([128, 128], F32, tag=f"o2{nh}")
                for c in range(4):
                    nc.tensor.matmul(
                        o2[:, 32 * c : 32 * c + 32],
                        s_sb[:, 512 * nh + 128 * c : 512 * nh + 128 * c + 128],
                        i32_sb[:, :],
                        start=(c == 0),
                        stop=(c == 3),
                        skip_group_check=True,
                    )
                tmp = tmp_pool.tile([128, 128], F32, tag=f"tmp{nh}")
                nc.vector.tensor_add(
                    tmp[:, :], o2[:, :], pr[:, 128 * nh : 128 * nh + 128]
                )
                nc.scalar.activation(
                    hT_next[:, 128 * nh : 128 * nh + 128],
                    tmp[:, :],
                    mybir.ActivationFunctionType.Relu,
                )
            if hT_store is not None:
                hT_store(t, hT_next)
            hT = hT_next
            if on_step is not None:
                on_step(t)
        return hT


def build_rnn(T):
    ntok = T * BL
    nc = bacc.Bacc("TRN2", target_bir_lowering=False, debug=False)

    xT_d = nc.dram_tensor("xT", [I_DIM, ntok], F32R, kind="ExternalInput").ap()
    wih0_d = nc.dram_tensor("wih0T", [128, KI * H], F32R, kind="ExternalInput").ap()
    whh0_d = nc.dram_tensor("whh0T", [128, KT * H], F32R, kind="ExternalInput").ap()
    wih1_d = nc.dram_tensor("wih1T", [128, KT * H], F32R, kind="ExternalInput").ap()
    whh1_d = nc.dram_tensor("whh1T", [128, KT * H], F32R, kind="ExternalInput").ap()
    fcw_d = nc.dram_tensor("fcwT", [128, KT * O], F32R, kind="ExternalInput").ap()
    bias0_d = nc.dram_tensor("bias0", [128, KT], F32, kind="ExternalInput").ap()
    bias1_d = nc.dram_tensor("bias1", [128, KT], F32, kind="ExternalInput").ap()
    fcb_d = nc.dram_tensor("fcb", [BL, O], F32, kind="ExternalInput").ap()
    i32_d = nc.dram_tensor("i32", [BL, BL], BF16, kind="ExternalInput").ap()
    zeros_d = nc.dram_tensor("zeros", [128, 2 * 128], F32R, kind="ExternalInput").ap()
    out_d = nc.dram_tensor("out", [BL, O], F32, kind="ExternalOutput").ap()

    with tile.TileContext(nc) as tc:
        with (
            tc.tile_pool(name="dram", bufs=1, space="DRAM") as dram_pool,
            tc.tile_pool(name="const", bufs=1) as cpool,
            tc.tile_pool(name="hT", bufs=2) as hT_pool,
        ):
            n_ch = max(T // 16, 1)
            ch = min(16, T)  # steps per chunk
            pre0_dram = dram_pool.tile([T, 128, 2 * 128], F32)
            pre1_ch = [dram_pool.tile([ch, 128, 2 * 128], F32, tag=f"p1_{i}",
                                      name=f"p1_{i}") for i in range(n_ch)]
            h0T_ch = [dram_pool.tile([ch, 128, 2 * 128], F32R, tag=f"h0_{i}",
                                     name=f"h0_{i}") for i in range(n_ch)]

            bias0_sb = cpool.tile([128, KT], F32)
            bias1_sb = cpool.tile([128, KT], F32)
            i32_sb = cpool.tile([BL, BL], BF16)
            fcb_sb = cpool.tile([BL, O], F32)
            nc.sync.dma_start(bias0_sb[:], bias0_d)
            nc.sync.dma_start(bias1_sb[:], bias1_d)
            nc.sync.dma_start(i32_sb[:], i32_d)
            nc.sync.dma_start(fcb_sb[:], fcb_d)

            # ---------- Phase A: pre0T ----------
            def load_x(tci, dest):
                nc.sync.dma_start(
                    dest[:, :].rearrange("p (ki n) -> p ki n", ki=KI),
                    xT_d[:, 512 * tci : 512 * tci + 512]
                    .rearrange("(ki p) n -> p ki n", p=128),
                )

            _phase_proj(nc, tc, ntok, KI, wih0_d, bias0_sb, load_x,
                        pre0_dram, "pA", rhs_bufs=9)

            # ---------- Phase B: layer-0 recurrence, phase C interleaved ----------
            def pre0_load(t, pr):
                nc.sync.dma_start(
                    pr[:, :].rearrange("p (t b) -> p t b", t=1),
                    pre0_dram[t : t + 1, :, :].rearrange("t p b -> p t b"),
                )

            def h0_store(t, hT_t):
                nc.sync.dma_start(
                    h0T_ch[t // ch][t % ch : t % ch + 1, :, :]
                    .rearrange("t p b -> p t b"),
                    hT_t[:, :].rearrange("p (t b) -> p t b", t=1),
                )

            with (
                tc.tile_pool(name="pC_w", bufs=1) as cw_pool,
                tc.tile_pool(name="pC_rhs", bufs=2) as crhs_pool,
                tc.tile_pool(name="pC_ps", bufs=2, space="PSUM") as cps_pool,
                tc.tile_pool(name="pC_st", bufs=3) as cst_pool,
            ):
                wih1_sb = cw_pool.tile([128, KT * H], F32R)
                nc.sync.dma_start(wih1_sb[:], wih1_d)

                def emit_c_chunk(ci):
                    rt = crhs_pool.tile([128, KT * 512], F32R, tag="crhs")
                    nc.sync.dma_start(
                        rt[:, :].rearrange("p (kt tt b) -> p kt tt b", kt=KT, b=BL),
                        h0T_ch[ci][:, :, :]
                        .rearrange("tt p (kt b) -> p kt tt b", b=BL),
                    )
                    for m in range(KT):
                        ps = cps_pool.tile([128, 512], F32, tag="cps")
                        for ki in range(KT):
                            nc.tensor.matmul(
                                ps[:, :],
                                wih1_sb[:, ki * H + 128 * m : ki * H + 128 * m + 128],
                                rt[:, ki * 512 : ki * 512 + 512],
                                start=(ki == 0),
                                stop=(ki == KT - 1),
                            )
                        st = cst_pool.tile([128, 512], F32, tag="cst")
                        nc.scalar.activation(
                            st[:, :], ps[:, :],
                            mybir.ActivationFunctionType.Identity,
                            bias=bias1_sb[:, m : m + 1],
                        )
                        nc.sync.dma_start(
                            pre1_ch[ci][:, :, 32 * m : 32 * m + 32]
                            .rearrange("t p b -> p t b"),
                            st[:, :].rearrange("p (t b) -> p t b", b=BL),
                        )

                def on_step(t):
                    if (t + 1) % ch == 0:
                        emit_c_chunk((t + 1) // ch - 1)

                _phase_recur(nc, tc, T, hT_pool, whh0_d, i32_sb, pre0_load,
                             h0_store, "pB", zeros_d, on_step=on_step)

            # ---------- Phase D: layer-1 recurrence ----------
            def pre1_load(t, pr):
                nc.sync.dma_start(
                    pr[:, :].rearrange("p (t b) -> p t b", t=1),
                    pre1_ch[t // ch][t % ch : t % ch + 1, :, :]
                    .rearrange("t p b -> p t b"),
                )

            hT_fin = _phase_recur(nc, tc, T, hT_pool, whh1_d, i32_sb,
                                  pre1_load, None, "pD", zeros_d)

            # ---------- Phase E: head ----------
            with (
                tc.tile_pool(name="fcw", bufs=1) as fpool,
                tc.tile_pool(name="eps", bufs=1, space="PSUM") as eps_pool,
                tc.tile_pool(name="eout", bufs=1) as eo_pool,
            ):
                fcw_sb = fpool.tile([128, KT * O], F32R)
                nc.sync.dma_start(fcw_sb[:], fcw_d)
                ep = eps_pool.tile([BL, O], F32)
                for kt in range(KT):
                    nc.tensor.matmul(
                        ep[:, :],
                        hT_fin[:, 32 * kt : 32 * kt + 32],
                        fcw_sb[:, kt * O : kt * O + O],
                        start=(kt == 0),
                        stop=(kt == KT - 1),
                    )
                eo = eo_pool.tile([BL, O], F32)
                nc.vector.tensor_add(eo[:, :], ep[:, :], fcb_sb[:, :])
                nc.sync.dma_start(out_d, eo[:, :])

    nc.compile()
    return nc


def _prep_core_inputs(inputs, T):
    """Host-side prep: transposed weights (shared) + per-core xT shards."""
    f32 = np.float32
    W_ih0 = np.asarray(inputs["W_ih0"], f32)
    W_hh0 = np.asarray(inputs["W_hh0"], f32)
    W_ih1 = np.asarray(inputs["W_ih1"], f32)
    W_hh1 = np.asarray(inputs["W_hh1"], f32)
    fc_w = np.asarray(inputs["fc_w"], f32)

    def stack_T(W, n_k):  # [128, n_k*cols]: [r, k*cols + j] = W[j, 128k + r]
        cols = W.shape[0]
        out = np.empty((128, n_k * cols), f32)
        WT = np.ascontiguousarray(W.T)  # [in, out]
        for k in range(n_k):
            out[:, k * cols : (k + 1) * cols] = WT[128 * k : 128 * (k + 1), :]
        return out

    shared = {
        "wih0T": stack_T(W_ih0, KI),
        "whh0T": stack_T(W_hh0, KT),
        "wih1T": stack_T(W_ih1, KT),
        "whh1T": stack_T(W_hh1, KT),
        "fcwT": stack_T(fc_w, KT),
        "bias0": np.ascontiguousarray(
            (np.asarray(inputs["b_ih0"], f32) + np.asarray(inputs["b_hh0"], f32))
            .reshape(KT, 128).T),
        "bias1": np.ascontiguousarray(
            (np.asarray(inputs["b_ih1"], f32) + np.asarray(inputs["b_hh1"], f32))
            .reshape(KT, 128).T),
        "fcb": np.tile(np.asarray(inputs["fc_b"], f32)[None, :], (BL, 1)),
        "i32": np.eye(BL, dtype=f32).astype(ml_dtypes.bfloat16),
        "zeros": np.zeros((128, 256), f32),
    }
    x = np.asarray(inputs["input_data"], f32)  # [B, T, I]
    in_maps = []
    for c in range(N_CORES):
        xs = x[c * BL : (c + 1) * BL, :T, :]          # [BL, T, I]
        xT = np.ascontiguousarray(np.transpose(xs, (2, 1, 0))).reshape(I_DIM, T * BL)
        in_maps.append(dict(shared, xT=xT))
    return in_maps


def run(inputs, trace=False, trace_kwargs=None):
    T = np.asarray(inputs["input_data"]).shape[1]
    nc = build_rnn(T)
    in_maps = _prep_core_inputs(inputs, T)
    res = run_bass_kernel_spmd(
        nc, in_maps, list(range(N_CORES)), trace=trace, **(trace_kwargs or {})
    )
    out = np.concatenate([res.results[c]["out"] for c in range(N_CORES)], axis=0)
    return out, res


def kernel(**inputs):
    return run(inputs)[0]


# revision 10
# speedup vs baseline: 1.1572x; 1.0528x over previous
"""Trainium2 Bass kernel for a 2-layer ReLU RNN (batch_first) + linear head.

Problem shapes: B=256, T=512, I=512, H=1024, O=256 (fp32).
Sharding: data-parallel over batch across 8 NeuronCores (32 rows each);
weights replicated. No collectives needed.

Per-core dataflow (all matmuls fp32r = full-rate reduced-precision fp32):
  Phase A: pre0T = W_ih0 @ x^T + biases       (stationary weight tiles, tokens stream)
  Phase B: layer-0 recurrence over T steps:
             state kept TRANSPOSED: hT[jj, kt*32+b] = h[b, 128*kt+jj]
             s = h @ W_hh.T via stationary-hT matmuls -> psum [32, 1024]
             hT_next = relu(transpose(s) + preT[t]); transpose on the PE
             (stationary = s-chunk [32,128], rhs = 32x32 identity)
             hT dumped to DRAM each step (input of phase C)
  Phase C: pre1T = W_ih1 @ h0^T + biases      (like phase A)
  Phase D: layer-1 recurrence (like B, no dump)
  Phase E: out = h1[:, -1, :] @ fc_w.T + fc_b

kernel(**inputs) takes the FULL unsharded inputs (keys as in reference
setup_inputs) and returns the FULL [256, 256] output.
"""

import ml_dtypes
import numpy as np

import concourse.bass as bass
import concourse.tile as tile
import concourse.mybir as mybir
from concourse import bacc
from concourse.bass_utils import run_bass_kernel_spmd

F32 = mybir.dt.float32
F32R = mybir.dt.float32r
BF16 = mybir.dt.bfloat16

B_FULL, T_FULL, I_DIM, H, O = 256, 512, 512, 1024, 256
N_CORES = 8
BL = B_FULL // N_CORES  # 32 batch rows per core
KI = I_DIM // 128       # 4  k-tiles of the input dim
KT = H // 128           # 8  k-tiles of the hidden dim


def _phase_proj(nc, tc, ntok, n_ki, w_d, bias_sb, rhs_load, pre_dram, name, rhs_bufs):
    """pre^T[j, tok] = sum_i W[j, i] * rhs[i, tok] + bias[j], streamed over
    512-token chunks, 8 psum banks in flight.

    w_d: DRAM [128, n_ki*H] fp32r, [r, ki*H + j] = W[j, 128*ki + r]
    rhs_load(tci, dest): emits DMA filling dest [128, n_ki*512] with
             rhs[128*ki + r, 512*tci + n] at [r, ki*512 + n]
    pre_dram: [T, 128, 256] fp32; [t, jj, 32*m + b] = pre[b, 128*m + jj]
    """
    n_tc = ntok // 512
    with (
        tc.tile_pool(name=f"{name}_w", bufs=1) as w_pool,
        tc.tile_pool(name=f"{name}_rhs", bufs=rhs_bufs) as rhs_pool,
        tc.tile_pool(name=f"{name}_ps", bufs=1, space="PSUM") as ps_pool,
        tc.tile_pool(name=f"{name}_st", bufs=4) as st_pool,
    ):
        w_sb = w_pool.tile([128, n_ki * H], F32R)
        nc.sync.dma_start(w_sb[:], w_d)
        for tcg in range(0, n_tc, 8):
            chunk = list(range(tcg, min(tcg + 8, n_tc)))
            rhs_tiles = []
            for tci in chunk:
                rt = rhs_pool.tile([128, n_ki * 512], F32R, tag="rhs")
                rhs_load(tci, rt)
                rhs_tiles.append(rt)
            for m in range(KT):
                for sl, (tci, rt) in enumerate(zip(chunk, rhs_tiles)):
                    ps = ps_pool.tile([128, 512], F32, tag=f"ps{sl}")
                    for ki in range(n_ki):
                        nc.tensor.matmul(
                            ps[:, :],
                            w_sb[:, ki * H + 128 * m : ki * H + 128 * m + 128],
                            rt[:, ki * 512 : ki * 512 + 512],
                            start=(ki == 0),
                            stop=(ki == n_ki - 1),
                        )
                    st = st_pool.tile([128, 512], F32, tag="st")
                    nc.scalar.activation(
                        st[:, :], ps[:, :],
                        mybir.ActivationFunctionType.Identity,
                        bias=bias_sb[:, m : m + 1],
                    )
                    # dest: pre_dram[t0 + tt, jj, 32*m + b], 16 t per chunk
                    t0 = tci * 512 // BL
                    nc.sync.dma_start(
                        pre_dram[t0 : t0 + 16, :, 32 * m : 32 * m + 32]
                        .rearrange("t p b -> p t b"),
                        st[:, :].rearrange("p (t b) -> p t b", b=BL),
                    )


def _phase_recur(nc, tc, T, hT_pool, whh_d, i32_sb, pre_load, hT_store, name, zeros_d, on_step=None):
    """Recurrence; returns final hT tile (allocated from caller-owned hT_pool).

    whh_d: DRAM [128, KT*H] fp32r, [r, kt*H + j] = W_hh[j, 128*kt + r]
    pre_load(t, pr): DMA step-t preT into pr [128, 256]
    hT_store(t, hT): optional per-step dump;  on_step(t): post-step hook
    """
    with (
        tc.tile_pool(name=f"{name}_w", bufs=1) as w_pool,
        tc.tile_pool(name=f"{name}_pr", bufs=4) as pr_pool,
        tc.tile_pool(name=f"{name}_ssb", bufs=3) as ssb_pool,
        tc.tile_pool(name=f"{name}_tmp", bufs=4) as tmp_pool,
        tc.tile_pool(name=f"{name}_ps", bufs=1, space="PSUM") as ps_pool,
        tc.tile_pool(name=f"{name}_ps2", bufs=1, space="PSUM") as ps2_pool,
    ):
        whh_sb = w_pool.tile([128, KT * H], F32R)
        nc.sync.dma_start(whh_sb[:], whh_d)
        hT = hT_pool.tile([128, 2 * 128], F32R, tag="hT")
        nc.sync.dma_start(hT[:, :], zeros_d)
        for t in range(T):
            pr = pr_pool.tile([128, 256], F32, tag="pr")
            pre_load(t, pr)
            s_sb = ssb_pool.tile([BL, H], BF16, tag="ssb")
            for nh in range(2):
                sp = ps_pool.tile([BL, 512], F32, tag=f"s{nh}")
                for kt in range(KT):
                    nc.tensor.matmul(
                        sp[:, :],
                        hT[:, 32 * kt : 32 * kt + 32],
                        whh_sb[:, kt * H + 512 * nh : kt * H + 512 * nh + 512],
                        start=(kt == 0),
                        stop=(kt == KT - 1),
                    )
                if nh == 0:
                    nc.vector.tensor_copy(s_sb[:, 512 * nh : 512 * nh + 512], sp[:, :])
                else:
                    nc.scalar.copy(s_sb[:, 512 * nh : 512 * nh + 512], sp[:, :])
            hT_next = hT_pool.tile([128, 2 * 128], F32R, tag="hT")
            for nh in range(2):
                o2 = ps2_pool.tile([128, 128], F32, tag=f"o2{nh}")
                for c in range(4):
                    nc.tensor.matmul(
                        o2[:, 32 * c : 32 * c + 32],
                        s_sb[:, 512 * nh + 128 * c : 512 * nh + 128 * c + 128],
                        i32_sb[:, :],
                        start=(c == 0),
                        stop=(c == 3),
                        skip_group_check=True,
                    )
                tmp = tmp_pool.tile([128, 128], F32, tag=f"tmp{nh}")
                nc.vector.tensor_add(
                    tmp[:, :], o2[:, :], pr[:, 128 * nh : 128 * nh + 128]
                )
                nc.scalar.activation(
                    hT_next[:, 128 * nh : 128 * nh + 128],
                    tmp[:, :],
                    mybir.ActivationFunctionType.Relu,
                )
            if hT_store is not None:
                hT_store(t, hT_next)
            hT = hT_next
            if on_step is not None:
                on_step(t)
        return hT


def build_rnn(T):
    ntok = T * BL
    nc = bacc.Bacc("TRN2", target_bir_lowering=False, debug=False)

    xT_d = nc.dram_tensor("xT", [I_DIM, ntok], F32R, kind="ExternalInput").ap()
    wih0_d = nc.dram_tensor("wih0T", [128, KI * H], F32R, kind="ExternalInput").ap()
    whh0_d = nc.dram_tensor("whh0T", [128, KT * H], F32R, kind="ExternalInput").ap()
    wih1_d = nc.dram_tensor("wih1T", [128, KT * H], F32R, kind="ExternalInput").ap()
    whh1_d = nc.dram_tensor("whh1T", [128, KT * H], F32R, kind="ExternalInput").ap()
    fcw_d = nc.dram_tensor("fcwT", [128, KT * O], F32R, kind="ExternalInput").ap()
    bias0_d = nc.dram_tensor("bias0", [128, KT], F32, kind="ExternalInput").ap()
    bias1_d = nc.dram_tensor("bias1", [128, KT], F32, kind="ExternalInput").ap()
    fcb_d = nc.dram_tensor("fcb", [BL, O], F32, kind="ExternalInput").ap()
    i32_d = nc.dram_tensor("i32", [BL, BL], BF16, kind="ExternalInput").ap()
    zeros_d = nc.dram_tensor("zeros", [128, 2 * 128], F32R, kind="ExternalInput").ap()
    out_d = nc.dram_tensor("out", [BL, O], F32, kind="ExternalOutput").ap()

    with tile.TileContext(nc) as tc:
        with (
            tc.tile_pool(name="dram", bufs=1, space="DRAM") as dram_pool,
            tc.tile_pool(name="const", bufs=1) as cpool,
            tc.tile_pool(name="hT", bufs=2) as hT_pool,
        ):
            n_ch = max(T // 16, 1)
            ch = min(16, T)  # steps per chunk
            pre0_ch = [dram_pool.tile([ch, 128, 2 * 128], F32, tag=f"p0_{i}",
                                      name=f"p0_{i}") for i in range(n_ch)]
            pre1_ch = [dram_pool.tile([ch, 128, 2 * 128], F32, tag=f"p1_{i}",
                                      name=f"p1_{i}") for i in range(n_ch)]
            h0T_ch = [dram_pool.tile([ch, 128, 2 * 128], F32R, tag=f"h0_{i}",
                                     name=f"h0_{i}") for i in range(n_ch)]

            bias0_sb = cpool.tile([128, KT], F32)
            bias1_sb = cpool.tile([128, KT], F32)
            i32_sb = cpool.tile([BL, BL], BF16)
            fcb_sb = cpool.tile([BL, O], F32)
            nc.sync.dma_start(bias0_sb[:], bias0_d)
            nc.sync.dma_start(bias1_sb[:], bias1_d)
            nc.sync.dma_start(i32_sb[:], i32_d)
            nc.sync.dma_start(fcb_sb[:], fcb_d)

            # ---------- Phase B: layer-0 recurrence, phases A+C interleaved ----------
            def pre0_load(t, pr):
                nc.sync.dma_start(
                    pr[:, :].rearrange("p (t b) -> p t b", t=1),
                    pre0_ch[t // ch][t % ch : t % ch + 1, :, :]
                    .rearrange("t p b -> p t b"),
                )

            def h0_store(t, hT_t):
                nc.sync.dma_start(
                    h0T_ch[t // ch][t % ch : t % ch + 1, :, :]
                    .rearrange("t p b -> p t b"),
                    hT_t[:, :].rearrange("p (t b) -> p t b", t=1),
                )

            with (
                tc.tile_pool(name="pA_w", bufs=1) as aw_pool,
                tc.tile_pool(name="pA_rhs", bufs=2) as arhs_pool,
                tc.tile_pool(name="pA_ps", bufs=1, space="PSUM") as aps_pool,
                tc.tile_pool(name="pA_st", bufs=3) as ast_pool,
                tc.tile_pool(name="pC_w", bufs=1) as cw_pool,
                tc.tile_pool(name="pC_rhs", bufs=2) as crhs_pool,
                tc.tile_pool(name="pC_ps", bufs=1, space="PSUM") as cps_pool,
                tc.tile_pool(name="pC_st", bufs=3) as cst_pool,
            ):
                wih0_sb = aw_pool.tile([128, KI * H], F32R)
                nc.sync.dma_start(wih0_sb[:], wih0_d)
                wih1_sb = cw_pool.tile([128, KT * H], F32R)
                nc.sync.dma_start(wih1_sb[:], wih1_d)

                def emit_a_chunk(ci):
                    rt = arhs_pool.tile([128, KI * 512], F32R, tag="arhs")
                    nc.sync.dma_start(
                        rt[:, :].rearrange("p (ki n) -> p ki n", ki=KI),
                        xT_d[:, 512 * ci : 512 * ci + 512]
                        .rearrange("(ki p) n -> p ki n", p=128),
                    )
                    for m in range(KT):
                        ps = aps_pool.tile([128, 512], F32, tag="aps")
                        for ki in range(KI):
                            nc.tensor.matmul(
                                ps[:, :],
                                wih0_sb[:, ki * H + 128 * m : ki * H + 128 * m + 128],
                                rt[:, ki * 512 : ki * 512 + 512],
                                start=(ki == 0),
                                stop=(ki == KI - 1),
                            )
                        st = ast_pool.tile([128, 512], F32, tag="ast")
                        nc.scalar.activation(
                            st[:, :], ps[:, :],
                            mybir.ActivationFunctionType.Identity,
                            bias=bias0_sb[:, m : m + 1],
                        )
                        nc.sync.dma_start(
                            pre0_ch[ci][:, :, 32 * m : 32 * m + 32]
                            .rearrange("t p b -> p t b"),
                            st[:, :].rearrange("p (t b) -> p t b", b=BL),
                        )

                def emit_c_chunk(ci):
                    rt = crhs_pool.tile([128, KT * 512], F32R, tag="crhs")
                    nc.sync.dma_start(
                        rt[:, :].rearrange("p (kt tt b) -> p kt tt b", kt=KT, b=BL),
                        h0T_ch[ci][:, :, :]
                        .rearrange("tt p (kt b) -> p kt tt b", b=BL),
                    )
                    for m in range(KT):
                        ps = cps_pool.tile([128, 512], F32, tag="cps")
                        for ki in range(KT):
                            nc.tensor.matmul(
                                ps[:, :],
                                wih1_sb[:, ki * H + 128 * m : ki * H + 128 * m + 128],
                                rt[:, ki * 512 : ki * 512 + 512],
                                start=(ki == 0),
                                stop=(ki == KT - 1),
                            )
                        st = cst_pool.tile([128, 512], F32, tag="cst")
                        nc.scalar.activation(
                            st[:, :], ps[:, :],
                            mybir.ActivationFunctionType.Identity,
                            bias=bias1_sb[:, m : m + 1],
                        )
                        nc.sync.dma_start(
                            pre1_ch[ci][:, :, 32 * m : 32 * m + 32]
                            .rearrange("t p b -> p t b"),
                            st[:, :].rearrange("p (t b) -> p t b", b=BL),
                        )

                emit_a_chunk(0)
                if n_ch > 1:
                    emit_a_chunk(1)

                def on_step(t):
                    if (t + 1) % ch == 0:
                        k = (t + 1) // ch
                        if k + 1 < n_ch:
                            emit_a_chunk(k + 1)
                        emit_c_chunk(k - 1)

                _phase_recur(nc, tc, T, hT_pool, whh0_d, i32_sb, pre0_load,
                             h0_store, "pB", zeros_d, on_step=on_step)

            # ---------- Phase D: layer-1 recurrence ----------
            def pre1_load(t, pr):
                nc.sync.dma_start(
                    pr[:, :].rearrange("p (t b) -> p t b", t=1),
                    pre1_ch[t // ch][t % ch : t % ch + 1, :, :]
                    .rearrange("t p b -> p t b"),
                )

            hT_fin = _phase_recur(nc, tc, T, hT_pool, whh1_d, i32_sb,
                                  pre1_load, None, "pD", zeros_d)

            # ---------- Phase E: head ----------
            with (
                tc.tile_pool(name="fcw", bufs=1) as fpool,
                tc.tile_pool(name="eps", bufs=1, space="PSUM") as eps_pool,
                tc.tile_pool(name="eout", bufs=1) as eo_pool,
            ):
                fcw_sb = fpool.tile([128, KT * O], F32R)
                nc.sync.dma_start(fcw_sb[:], fcw_d)
                ep = eps_pool.tile([BL, O], F32)
                for kt in range(KT):
                    nc.tensor.matmul(
                        ep[:, :],
                        hT_fin[:, 32 * kt : 32 * kt + 32],
                        fcw_sb[:, kt * O : kt * O + O],
                        start=(kt == 0),
                        stop=(kt == KT - 1),
                    )
                eo = eo_pool.tile([BL, O], F32)
                nc.vector.tensor_add(eo[:, :], ep[:, :], fcb_sb[:, :])
                nc.sync.dma_start(out_d, eo[:, :])

    nc.compile()
    return nc


def _prep_core_inputs(inputs, T):
    """Host-side prep: transposed weights (shared) + per-core xT shards."""
    f32 = np.float32
    W_ih0 = np.asarray(inputs["W_ih0"], f32)
    W_hh0 = np.asarray(inputs["W_hh0"], f32)
    W_ih1 = np.asarray(inputs["W_ih1"], f32)
    W_hh1 = np.asarray(inputs["W_hh1"], f32)
    fc_w = np.asarray(inputs["fc_w"], f32)

    def stack_T(W, n_k):  # [128, n_k*cols]: [r, k*cols + j] = W[j, 128k + r]
        cols = W.shape[0]
        out = np.empty((128, n_k * cols), f32)
        WT = np.ascontiguousarray(W.T)  # [in, out]
        for k in range(n_k):
            out[:, k * cols : (k + 1) * cols] = WT[128 * k : 128 * (k + 1), :]
        return out

    shared = {
        "wih0T": stack_T(W_ih0, KI),
        "whh0T": stack_T(W_hh0, KT),
        "wih1T": stack_T(W_ih1, KT),
        "whh1T": stack_T(W_hh1, KT),
        "fcwT": stack_T(fc_w, KT),
        "bias0": np.ascontiguousarray(
            (np.asarray(inputs["b_ih0"], f32) + np.asarray(inputs["b_hh0"], f32))
            .reshape(KT, 128).T),
        "bias1": np.ascontiguousarray(
            (np.asarray(inputs["b_ih1"], f32) + np.asarray(inputs["b_hh1"], f32))
            .reshape(KT, 128).T),
        "fcb": np.tile(np.asarray(inputs["fc_b"], f32)[None, :], (BL, 1)),
        "i32": np.eye(BL, dtype=f32).astype(ml_dtypes.bfloat16),
        "zeros": np.zeros((128, 256), f32),
    }
    x = np.asarray(inputs["input_data"], f32)  # [B, T, I]
    in_maps = []
    for c in range(N_CORES):
        xs = x[c * BL : (c + 1) * BL, :T, :]          # [BL, T, I]
        xT = np.ascontiguousarray(np.transpose(xs, (2, 1, 0))).reshape(I_DIM, T * BL)
        in_maps.append(dict(shared, xT=xT))
    return in_maps


def run(inputs, trace=False, trace_kwargs=None):
    T = np.asarray(inputs["input_data"]).shape[1]
    nc = build_rnn(T)
    in_maps = _prep_core_inputs(inputs, T)
    res = run_bass_kernel_spmd(
        nc, in_maps, list(range(N_CORES)), trace=trace, **(trace_kwargs or {})
    )
    out = np.concatenate([res.results[c]["out"] for c in range(N_CORES)], axis=0)
    return out, res


def kernel(**inputs):
    return run(inputs)[0]


# revision 12
# speedup vs baseline: 1.2779x; 1.1043x over previous
"""Trainium2 Bass kernel for a 2-layer ReLU RNN (batch_first) + linear head.

Problem shapes: B=256, T=512, I=512, H=1024, O=256 (fp32).
Sharding: data-parallel over batch across 8 NeuronCores (32 rows each);
weights replicated. No collectives needed.

Per-core dataflow (all matmuls fp32r = full-rate reduced-precision fp32):
  Phase A: pre0T = W_ih0 @ x^T + biases       (stationary weight tiles, tokens stream)
  Phase B: layer-0 recurrence over T steps:
             state kept TRANSPOSED: hT[jj, kt*32+b] = h[b, 128*kt+jj]
             s = h @ W_hh.T via stationary-hT matmuls -> psum [32, 1024]
             hT_next = relu(transpose(s) + preT[t]); transpose on the PE
             (stationary = s-chunk [32,128], rhs = 32x32 identity)
             hT dumped to DRAM each step (input of phase C)
  Phase C: pre1T = W_ih1 @ h0^T + biases      (like phase A)
  Phase D: layer-1 recurrence (like B, no dump)
  Phase E: out = h1[:, -1, :] @ fc_w.T + fc_b

Phases A and C are emitted in 16-step chunks interleaved into phase B's
emission (chunked DRAM tiles carry the dependencies), so their matmuls fill
the recurrence's PE idle slots. Measured: 6.25 ms HW exec, 1.0e-3 rel err.

kernel(**inputs) takes the FULL unsharded inputs (keys as in reference
setup_inputs) and returns the FULL [256, 256] output.
"""

import ml_dtypes
import numpy as np

import concourse.bass as bass
import concourse.tile as tile
import concourse.mybir as mybir
from concourse import bacc
from concourse.bass_utils import run_bass_kernel_spmd

F32 = mybir.dt.float32
F32R = mybir.dt.float32r
BF16 = mybir.dt.bfloat16

B_FULL, T_FULL, I_DIM, H, O = 256, 512, 512, 1024, 256
N_CORES = 8
BL = B_FULL // N_CORES  # 32 batch rows per core
KI = I_DIM // 128       # 4  k-tiles of the input dim
KT = H // 128           # 8  k-tiles of the hidden dim


def _phase_proj(nc, tc, ntok, n_ki, w_d, bias_sb, rhs_load, pre_dram, name, rhs_bufs):
    """pre^T[j, tok] = sum_i W[j, i] * rhs[i, tok] + bias[j], streamed over
    512-token chunks, 8 psum banks in flight.

    w_d: DRAM [128, n_ki*H] fp32r, [r, ki*H + j] = W[j, 128*ki + r]
    rhs_load(tci, dest): emits DMA filling dest [128, n_ki*512] with
             rhs[128*ki + r, 512*tci + n] at [r, ki*512 + n]
    pre_dram: [T, 128, 256] fp32; [t, jj, 32*m + b] = pre[b, 128*m + jj]
    """
    n_tc = ntok // 512
    with (
        tc.tile_pool(name=f"{name}_w", bufs=1) as w_pool,
        tc.tile_pool(name=f"{name}_rhs", bufs=rhs_bufs) as rhs_pool,
        tc.tile_pool(name=f"{name}_ps", bufs=1, space="PSUM") as ps_pool,
        tc.tile_pool(name=f"{name}_st", bufs=4) as st_pool,
    ):
        w_sb = w_pool.tile([128, n_ki * H], F32R)
        nc.sync.dma_start(w_sb[:], w_d)
        for tcg in range(0, n_tc, 8):
            chunk = list(range(tcg, min(tcg + 8, n_tc)))
            rhs_tiles = []
            for tci in chunk:
                rt = rhs_pool.tile([128, n_ki * 512], F32R, tag="rhs")
                rhs_load(tci, rt)
                rhs_tiles.append(rt)
            for m in range(KT):
                for sl, (tci, rt) in enumerate(zip(chunk, rhs_tiles)):
                    ps = ps_pool.tile([128, 512], F32, tag=f"ps{sl}")
                    for ki in range(n_ki):
                        nc.tensor.matmul(
                            ps[:, :],
                            w_sb[:, ki * H + 128 * m : ki * H + 128 * m + 128],
                            rt[:, ki * 512 : ki * 512 + 512],
                            start=(ki == 0),
                            stop=(ki == n_ki - 1),
                        )
                    st = st_pool.tile([128, 512], F32, tag="st")
                    nc.scalar.activation(
                        st[:, :], ps[:, :],
                        mybir.ActivationFunctionType.Identity,
                        bias=bias_sb[:, m : m + 1],
                    )
                    # dest: pre_dram[t0 + tt, jj, 32*m + b], 16 t per chunk
                    t0 = tci * 512 // BL
                    nc.sync.dma_start(
                        pre_dram[t0 : t0 + 16, :, 32 * m : 32 * m + 32]
                        .rearrange("t p b -> p t b"),
                        st[:, :].rearrange("p (t b) -> p t b", b=BL),
                    )


def _phase_recur(nc, tc, T, hT_pool, whh_d, i32_sb, pre_load, hT_store, name, zeros_d, on_step=None):
    """Recurrence; returns final hT tile (allocated from caller-owned hT_pool).

    whh_d: DRAM [128, KT*H] fp32r, [r, kt*H + j] = W_hh[j, 128*kt + r]
    pre_load(t, pr): DMA step-t preT into pr [128, 256]
    hT_store(t, hT): optional per-step dump;  on_step(t): post-step hook
    """
    with (
        tc.tile_pool(name=f"{name}_w", bufs=1) as w_pool,
        tc.tile_pool(name=f"{name}_pr", bufs=4) as pr_pool,
        tc.tile_pool(name=f"{name}_ssb", bufs=3) as ssb_pool,
        tc.tile_pool(name=f"{name}_tmp", bufs=4) as tmp_pool,
        tc.tile_pool(name=f"{name}_ps", bufs=1, space="PSUM") as ps_pool,
        tc.tile_pool(name=f"{name}_ps2", bufs=1, space="PSUM") as ps2_pool,
    ):
        whh_sb = w_pool.tile([128, KT * H], F32R)
        nc.sync.dma_start(whh_sb[:], whh_d)
        hT = hT_pool.tile([128, 2 * 128], F32R, tag="hT")
        nc.sync.dma_start(hT[:, :], zeros_d)
        for t in range(T):
            pr = pr_pool.tile([128, 256], F32, tag="pr")
            pre_load(t, pr)
            s_sb = ssb_pool.tile([BL, H], BF16, tag="ssb")
            for nh in range(2):
                sp = ps_pool.tile([BL, 512], F32, tag=f"s{nh}")
                for kt in range(KT):
                    nc.tensor.matmul(
                        sp[:, :],
                        hT[:, 32 * kt : 32 * kt + 32],
                        whh_sb[:, kt * H + 512 * nh : kt * H + 512 * nh + 512],
                        start=(kt == 0),
                        stop=(kt == KT - 1),
                    )
                if nh == 0:
                    nc.vector.tensor_copy(s_sb[:, 512 * nh : 512 * nh + 512], sp[:, :])
                else:
                    nc.scalar.copy(s_sb[:, 512 * nh : 512 * nh + 512], sp[:, :])
            hT_next = hT_pool.tile([128, 2 * 128], F32R, tag="hT")
            for nh in range(2):
                o2 = ps2_pool.tile([128, 128], F32, tag=f"o2{nh}")
                for c in range(4):
                    nc.tensor.matmul(
                        o2[:, 32 * c : 32 * c + 32],
                        s_sb[:, 512 * nh + 128 * c : 512 * nh + 128 * c + 128],
                        i32_sb[:, :],
                        start=(c == 0),
                        stop=(c == 3),
                        skip_group_check=True,
                    )
                tmp = tmp_pool.tile([128, 128], F32, tag=f"tmp{nh}")
                nc.vector.tensor_add(
                    tmp[:, :], o2[:, :], pr[:, 128 * nh : 128 * nh + 128]
                )
                nc.scalar.activation(
                    hT_next[:, 128 * nh : 128 * nh + 128],
                    tmp[:, :],
                    mybir.ActivationFunctionType.Relu,
                )
            if hT_store is not None:
                hT_store(t, hT_next)
            hT = hT_next
            if on_step is not None:
                on_step(t)
        return hT


def build_rnn(T):
    ntok = T * BL
    nc = bacc.Bacc("TRN2", target_bir_lowering=False, debug=False)

    xT_d = nc.dram_tensor("xT", [I_DIM, ntok], F32R, kind="ExternalInput").ap()
    wih0_d = nc.dram_tensor("wih0T", [128, KI * H], F32R, kind="ExternalInput").ap()
    whh0_d = nc.dram_tensor("whh0T", [128, KT * H], F32R, kind="ExternalInput").ap()
    wih1_d = nc.dram_tensor("wih1T", [128, KT * H], F32R, kind="ExternalInput").ap()
    whh1_d = nc.dram_tensor("whh1T", [128, KT * H], F32R, kind="ExternalInput").ap()
    fcw_d = nc.dram_tensor("fcwT", [128, KT * O], F32R, kind="ExternalInput").ap()
    bias0_d = nc.dram_tensor("bias0", [128, KT], F32, kind="ExternalInput").ap()
    bias1_d = nc.dram_tensor("bias1", [128, KT], F32, kind="ExternalInput").ap()
    fcb_d = nc.dram_tensor("fcb", [BL, O], F32, kind="ExternalInput").ap()
    i32_d = nc.dram_tensor("i32", [BL, BL], BF16, kind="ExternalInput").ap()
    zeros_d = nc.dram_tensor("zeros", [128, 2 * 128], F32R, kind="ExternalInput").ap()
    out_d = nc.dram_tensor("out", [BL, O], F32, kind="ExternalOutput").ap()

    with tile.TileContext(nc) as tc:
        with (
            tc.tile_pool(name="dram", bufs=1, space="DRAM") as dram_pool,
            tc.tile_pool(name="const", bufs=1) as cpool,
            tc.tile_pool(name="hT", bufs=2) as hT_pool,
        ):
            n_ch = max(T // 16, 1)
            ch = min(16, T)  # steps per chunk
            pre0_ch = [dram_pool.tile([ch, 128, 2 * 128], F32, tag=f"p0_{i}",
                                      name=f"p0_{i}") for i in range(n_ch)]
            pre1_ch = [dram_pool.tile([ch, 128, 2 * 128], F32, tag=f"p1_{i}",
                                      name=f"p1_{i}") for i in range(n_ch)]
            h0T_ch = [dram_pool.tile([ch, 128, 2 * 128], F32R, tag=f"h0_{i}",
                                     name=f"h0_{i}") for i in range(n_ch)]

            bias0_sb = cpool.tile([128, KT], F32)
            bias1_sb = cpool.tile([128, KT], F32)
            i32_sb = cpool.tile([BL, BL], BF16)
            fcb_sb = cpool.tile([BL, O], F32)
            nc.sync.dma_start(bias0_sb[:], bias0_d)
            nc.sync.dma_start(bias1_sb[:], bias1_d)
            nc.sync.dma_start(i32_sb[:], i32_d)
            nc.sync.dma_start(fcb_sb[:], fcb_d)

            # ---------- Phase B: layer-0 recurrence, phases A+C interleaved ----------
            def pre0_load(t, pr):
                nc.sync.dma_start(
                    pr[:, :].rearrange("p (t b) -> p t b", t=1),
                    pre0_ch[t // ch][t % ch : t % ch + 1, :, :]
                    .rearrange("t p b -> p t b"),
                )

            def h0_store(t, hT_t):
                nc.sync.dma_start(
                    h0T_ch[t // ch][t % ch : t % ch + 1, :, :]
                    .rearrange("t p b -> p t b"),
                    hT_t[:, :].rearrange("p (t b) -> p t b", t=1),
                )

            with (
                tc.tile_pool(name="pA_w", bufs=1) as aw_pool,
                tc.tile_pool(name="pA_rhs", bufs=2) as arhs_pool,
                tc.tile_pool(name="pA_ps", bufs=1, space="PSUM") as aps_pool,
                tc.tile_pool(name="pA_st", bufs=3) as ast_pool,
                tc.tile_pool(name="pC_w", bufs=1) as cw_pool,
                tc.tile_pool(name="pC_rhs", bufs=2) as crhs_pool,
                tc.tile_pool(name="pC_ps", bufs=1, space="PSUM") as cps_pool,
                tc.tile_pool(name="pC_st", bufs=3) as cst_pool,
            ):
                wih0_sb = aw_pool.tile([128, KI * H], F32R)
                nc.sync.dma_start(wih0_sb[:], wih0_d)
                wih1_sb = cw_pool.tile([128, KT * H], F32R)
                nc.sync.dma_start(wih1_sb[:], wih1_d)

                def emit_a_chunk(ci):
                    rt = arhs_pool.tile([128, KI * 512], F32R, tag="arhs")
                    nc.sync.dma_start(
                        rt[:, :].rearrange("p (ki n) -> p ki n", ki=KI),
                        xT_d[:, 512 * ci : 512 * ci + 512]
                        .rearrange("(ki p) n -> p ki n", p=128),
                    )
                    for m in range(KT):
                        ps = aps_pool.tile([128, 512], F32, tag="aps")
                        for ki in range(KI):
                            nc.tensor.matmul(
                                ps[:, :],
                                wih0_sb[:, ki * H + 128 * m : ki * H + 128 * m + 128],
                                rt[:, ki * 512 : ki * 512 + 512],
                                start=(ki == 0),
                                stop=(ki == KI - 1),
                            )
                        st = ast_pool.tile([128, 512], F32, tag="ast")
                        nc.scalar.activation(
                            st[:, :], ps[:, :],
                            mybir.ActivationFunctionType.Identity,
                            bias=bias0_sb[:, m : m + 1],
                        )
                        nc.sync.dma_start(
                            pre0_ch[ci][:, :, 32 * m : 32 * m + 32]
                            .rearrange("t p b -> p t b"),
                            st[:, :].rearrange("p (t b) -> p t b", b=BL),
                        )

                def emit_c_chunk(ci):
                    rt = crhs_pool.tile([128, KT * 512], F32R, tag="crhs")
                    nc.sync.dma_start(
                        rt[:, :].rearrange("p (kt tt b) -> p kt tt b", kt=KT, b=BL),
                        h0T_ch[ci][:, :, :]
                        .rearrange("tt p (kt b) -> p kt tt b", b=BL),
                    )
                    for m in range(KT):
                        ps = cps_pool.tile([128, 512], F32, tag="cps")
                        for ki in range(KT):
                            nc.tensor.matmul(
                                ps[:, :],
                                wih1_sb[:, ki * H + 128 * m : ki * H + 128 * m + 128],
                                rt[:, ki * 512 : ki * 512 + 512],
                                start=(ki == 0),
                                stop=(ki == KT - 1),
                            )
                        st = cst_pool.tile([128, 512], F32, tag="cst")
                        nc.scalar.activation(
                            st[:, :], ps[:, :],
                            mybir.ActivationFunctionType.Identity,
                            bias=bias1_sb[:, m : m + 1],
                        )
                        nc.sync.dma_start(
                            pre1_ch[ci][:, :, 32 * m : 32 * m + 32]
                            .rearrange("t p b -> p t b"),
                            st[:, :].rearrange("p (t b) -> p t b", b=BL),
                        )

                emit_a_chunk(0)
                if n_ch > 1:
                    emit_a_chunk(1)

                def on_step(t):
                    if (t + 1) % ch == 0:
                        k = (t + 1) // ch
                        if k + 1 < n_ch:
                            emit_a_chunk(k + 1)

                _phase_recur(nc, tc, T, hT_pool, whh0_d, i32_sb, pre0_load,
                             h0_store, "pB", zeros_d, on_step=on_step)

                # C chunks 0..1 before phase D starts; the rest interleave into D
                emit_c_chunk(0)
                if n_ch > 1:
                    emit_c_chunk(1)

                def on_step_d(t):
                    if (t + 1) % ch == 0:
                        k = (t + 1) // ch
                        if k + 1 < n_ch:
                            emit_c_chunk(k + 1)

                # ---------- Phase D: layer-1 recurrence, C interleaved ----------
                def pre1_load(t, pr):
                    nc.sync.dma_start(
                        pr[:, :].rearrange("p (t b) -> p t b", t=1),
                        pre1_ch[t // ch][t % ch : t % ch + 1, :, :]
                        .rearrange("t p b -> p t b"),
                    )

                hT_fin = _phase_recur(nc, tc, T, hT_pool, whh1_d, i32_sb,
                                      pre1_load, None, "pD", zeros_d,
                                      on_step=on_step_d)

            # ---------- Phase E: head ----------
            with (
                tc.tile_pool(name="fcw", bufs=1) as fpool,
                tc.tile_pool(name="eps", bufs=1, space="PSUM") as eps_pool,
                tc.tile_pool(name="eout", bufs=1) as eo_pool,
            ):
                fcw_sb = fpool.tile([128, KT * O], F32R)
                nc.sync.dma_start(fcw_sb[:], fcw_d)
                ep = eps_pool.tile([BL, O], F32)
                for kt in range(KT):
                    nc.tensor.matmul(
                        ep[:, :],
                        hT_fin[:, 32 * kt : 32 * kt + 32],
                        fcw_sb[:, kt * O : kt * O + O],
                        start=(kt == 0),
                        stop=(kt == KT - 1),
                    )
                eo = eo_pool.tile([BL, O], F32)
                nc.vector.tensor_add(eo[:, :], ep[:, :], fcb_sb[:, :])
                nc.sync.dma_start(out_d, eo[:, :])

    nc.compile()
    return nc


def _prep_core_inputs(inputs, T):
    """Host-side prep: transposed weights (shared) + per-core xT shards."""
    f32 = np.float32
    W_ih0 = np.asarray(inputs["W_ih0"], f32)
    W_hh0 = np.asarray(inputs["W_hh0"], f32)
    W_ih1 = np.asarray(inputs["W_ih1"], f32)
    W_hh1 = np.asarray(inputs["W_hh1"], f32)
    fc_w = np.asarray(inputs["fc_w"], f32)

    def stack_T(W, n_k):  # [128, n_k*cols]: [r, k*cols + j] = W[j, 128k + r]
        cols = W.shape[0]
        out = np.empty((128, n_k * cols), f32)
        WT = np.ascontiguousarray(W.T)  # [in, out]
        for k in range(n_k):
            out[:, k * cols : (k + 1) * cols] = WT[128 * k : 128 * (k + 1), :]
        return out

    shared = {
        "wih0T": stack_T(W_ih0, KI),
        "whh0T": stack_T(W_hh0, KT),
        "wih1T": stack_T(W_ih1, KT),
        "whh1T": stack_T(W_hh1, KT),
        "fcwT": stack_T(fc_w, KT),
        "bias0": np.ascontiguousarray(
            (np.asarray(inputs["b_ih0"], f32) + np.asarray(inputs["b_hh0"], f32))
            .reshape(KT, 128).T),
        "bias1": np.ascontiguousarray(
            (np.asarray(inputs["b_ih1"], f32) + np.asarray(inputs["b_hh1"], f32))
            .reshape(KT, 128).T),
        "fcb": np.tile(np.asarray(inputs["fc_b"], f32)[None, :], (BL, 1)),
        "i32": np.eye(BL, dtype=f32).astype(ml_dtypes.bfloat16),
        "zeros": np.zeros((128, 256), f32),
    }
    x = np.asarray(inputs["input_data"], f32)  # [B, T, I]
    in_maps = []
    for c in range(N_CORES):
        xs = x[c * BL : (c + 1) * BL, :T, :]          # [BL, T, I]
        xT = np.ascontiguousarray(np.transpose(xs, (2, 1, 0))).reshape(I_DIM, T * BL)
        in_maps.append(dict(shared, xT=xT))
    return in_maps


def run(inputs, trace=False, trace_kwargs=None):
    T = np.asarray(inputs["input_data"]).shape[1]
    nc = build_rnn(T)
    in_maps = _prep_core_inputs(inputs, T)
    res = run_bass_kernel_spmd(
        nc, in_maps, list(range(N_CORES)), trace=trace, **(trace_kwargs or {})
    )
    out = np.concatenate([res.results[c]["out"] for c in range(N_CORES)], axis=0)
    return out, res


def kernel(**inputs):
    return run(inputs)[0]
